# revision 5
# baseline (speedup 1.0000x reference)
"""Trainium2 Bass kernel for masked attention with pre-softmax-score AV matmul.

Reference semantics (faithful to the source module's bug):
    a = (Q @ K^T) / sqrt(D);  a = where(mask, -1e9, a)
    attn_p = softmax(a, axis=-1)
    attn_v = a @ V            # uses pre-softmax masked scores, NOT attn_p
    returns (attn_v, attn_p)

Shapes: Q,K,V (2,8,2048,64) f32; mask (2,8,2048,2048) bool.
Sharding: B*H = 16 head-slices, 2 per core across 8 cores (pure data
parallel, no collectives).
"""

import sys

sys.path.insert(0, "/opt/trn_rl_repo")

import numpy as np

import concourse.bass as bass
import concourse.tile as tile
from concourse import mybir
from concourse.bass_utils import run_bass_kernel_spmd
from concourse.masks import make_identity

B, H, S, D = 2, 8, 2048, 64
N_CORES = 8
HEADS_PER_CORE = (B * H) // N_CORES  # 2
P = 128                              # partition tile (q rows per tile)
NQT = S // P                         # 16 q-tiles per head
KC = S // P                          # 16 k-chunks of 128
GRP = 4                              # q-tiles per AV matmul group
SCALE = 1.0 / np.sqrt(np.float32(D))  # 0.125, exact in fp32
NEG = -1.0e9

f32 = mybir.dt.float32
f32r = mybir.dt.float32r
bf16 = mybir.dt.bfloat16
u8 = mybir.dt.uint8

_COMPILED = {}


def legalize_waits(nc):
    """This walrus build allows only ONE sync-wait command per instruction.

    Tile's wait assigner can attach several (one per upstream engine), which
    fails codegen with 'Too many sync wait commands'. Hoist all but the last
    wait onto preceding same-engine NoOps (program order on the engine's
    sequencer makes this semantically identical).
    """
    n_fixed = 0
    for fn in nc.m.functions:
        for blk in fn.blocks:
            insts = list(blk.instructions)
            new_list = []
            changed = False
            for inst in insts:
                si = inst.sync_info
                if si is not None and si.on_wait is not None and len(si.on_wait) > 1:
                    waits = list(si.on_wait)
                    for wi, w in enumerate(waits[:-1]):
                        new_list.append(
                            mybir.InstNoOp(
                                name=f"{inst.name}-wn{wi}",
                                engine=inst.engine,
                                sync_info=mybir.SyncInfo(on_wait=[w], on_update=[]),
                            )
                        )
                    inst.sync_info = mybir.SyncInfo(
                        on_wait=waits[-1:], on_update=list(si.on_update or [])
                    )
                    changed = True
                    n_fixed += 1
                new_list.append(inst)
            if changed:
                blk.instructions = new_list
    return n_fixed


def build_bass():
    nc = bass.Bass()

    q_ext = nc.declare_dram_parameter("q", [HEADS_PER_CORE, S, D], f32, isOutput=False)
    k_ext = nc.declare_dram_parameter("k", [HEADS_PER_CORE, S, D], f32, isOutput=False)
    v_ext = nc.declare_dram_parameter("v", [HEADS_PER_CORE, S, D], f32, isOutput=False)
    m_ext = nc.declare_dram_parameter("m", [HEADS_PER_CORE, S, S], u8, isOutput=False)
    outv = nc.declare_dram_parameter("out_v", [HEADS_PER_CORE, S, D], f32, isOutput=True)
    outp = nc.declare_dram_parameter("out_p", [HEADS_PER_CORE, S, S], f32, isOutput=True)

    with tile.TileContext(nc) as tc:
        with (
            tc.tile_pool(name="const", bufs=1) as const_pool,
            tc.tile_pool(name="head", bufs=2) as head_pool,
            tc.tile_pool(name="mask", bufs=3) as mask_pool,
            tc.tile_pool(name="sc", bufs=2) as sc_pool,
            tc.tile_pool(name="pn", bufs=2) as pn_pool,
            tc.tile_pool(name="at", bufs=2) as at_pool,
            tc.tile_pool(name="small", bufs=4) as small_pool,
            tc.tile_pool(name="vo", bufs=4) as vo_pool,
            tc.tile_pool(name="ps_qk", bufs=1, space="PSUM") as ps_qk,
            tc.tile_pool(name="ps_tr", bufs=2, space="PSUM") as ps_tr,
            tc.tile_pool(name="ps_av", bufs=2, space="PSUM") as ps_av,
        ):
            ident = const_pool.tile([P, P], f32)
            make_identity(nc, ident)

            for h in range(HEADS_PER_CORE):
                # ---- per-head prep: QT/KT [64, S] f32 (Q scaled), V bf16 ----
                q_nat = head_pool.tile([P, KC, D], f32, tag="q_nat")
                k_nat = head_pool.tile([P, KC, D], f32, tag="k_nat")
                v_nat = head_pool.tile([P, KC, D], f32, tag="v_nat")
                nc.sync.dma_start(
                    out=q_nat, in_=q_ext[h].rearrange("(t p) d -> p t d", p=P)
                )
                nc.sync.dma_start(
                    out=k_nat, in_=k_ext[h].rearrange("(t p) d -> p t d", p=P)
                )
                nc.sync.dma_start(
                    out=v_nat, in_=v_ext[h].rearrange("(t p) d -> p t d", p=P)
                )

                qt = head_pool.tile([D, S], f32r, tag="qt")
                kt = head_pool.tile([D, S], f32r, tag="kt")
                v_bf = head_pool.tile([P, KC, D], bf16, tag="v_bf")
                nc.vector.tensor_copy(v_bf, v_nat)

                for t in range(KC):
                    pt = ps_tr.tile([P, 512], f32, tag="ps_tr")
                    nc.tensor.transpose(pt[:D, :P], q_nat[:, t, :], ident)
                    nc.tensor.transpose(pt[:D, P : 2 * P], k_nat[:, t, :], ident)
                    # fold the 1/sqrt(D) scale into Q^T
                    nc.scalar.activation(
                        qt[:, t * P : (t + 1) * P],
                        pt[:D, :P],
                        mybir.ActivationFunctionType.Copy,
                        scale=float(SCALE),
                    )
                    nc.scalar.copy(kt[:, t * P : (t + 1) * P], pt[:D, P : 2 * P])

                for g in range(NQT // GRP):
                    # corner-turn buffer: aT_g[p, kc, q_in_group]
                    at_g = at_pool.tile([P, KC, GRP * P], bf16, tag="at_g")

                    for gq in range(GRP):
                        qi = g * GRP + gq
                        qs = qi * P

                        m_tile = mask_pool.tile([P, S], u8, tag="m")
                        nc.sync.dma_start(out=m_tile, in_=m_ext[h, qs : qs + P, :])

                        qk = ps_qk.tile([P, S], f32, tag="qk")
                        for kj in range(4):
                            nc.tensor.matmul(
                                qk[:, kj * 512 : (kj + 1) * 512],
                                qt[:, qs : qs + P],
                                kt[:, kj * 512 : (kj + 1) * 512],
                                start=True,
                                stop=True,
                            )

                        # a = qk + mask * (-1e9)   (masked scores, f32)
                        a_t = sc_pool.tile([P, S], f32, tag="a")
                        nc.vector.scalar_tensor_tensor(
                            out=a_t,
                            in0=m_tile,
                            scalar=NEG,
                            in1=qk,
                            op0=mybir.AluOpType.mult,
                            op1=mybir.AluOpType.add,
                        )

                        # p_unnorm = exp(a), rowsum via accum
                        p_un = pn_pool.tile([P, S], f32, tag="p_un")
                        rsum = small_pool.tile([P, 1], f32, tag="rsum")
                        nc.scalar.activation(
                            p_un,
                            a_t,
                            mybir.ActivationFunctionType.Exp,
                            accum_out=rsum,
                        )
                        rinv = small_pool.tile([P, 1], f32, tag="rinv")
                        nc.vector.reciprocal(rinv, rsum)

                        p_out = pn_pool.tile([P, S], f32, tag="p_out")
                        nc.vector.tensor_scalar_mul(p_out, p_un, rinv)
                        nc.sync.dma_start(out=outp[h, qs : qs + P, :], in_=p_out)

                        # transpose a into the corner-turn buffer (bf16)
                        for kq in range(4):
                            pt = ps_tr.tile([P, 512], f32, tag="ps_tr")
                            for kk in range(4):
                                kj = kq * 4 + kk
                                nc.tensor.transpose(
                                    pt[:, kk * P : (kk + 1) * P],
                                    a_t[:, kj * P : (kj + 1) * P],
                                    ident,
                                )
                            nc.scalar.copy(
                                at_g[:, kq * 4 : kq * 4 + 4, gq * P : (gq + 1) * P],
                                pt.rearrange("p (a b) -> p a b", a=4),
                            )

                    # ---- AV for this group: attn_vT[64, GRP*P] ----
                    av = ps_av.tile([D, GRP * P], f32, tag="av")
                    for kj in range(KC):
                        nc.tensor.matmul(
                            av,
                            v_bf[:, kj, :],
                            at_g[:, kj, :],
                            start=(kj == 0),
                            stop=(kj == KC - 1),
                        )
                    av_sb = vo_pool.tile([D, GRP * P], f32, tag="av_sb")
                    nc.vector.tensor_copy(av_sb, av)
                    # transpose back to [q, D] and write out
                    for gq in range(GRP):
                        pt2 = ps_tr.tile([P, 512], f32, tag="ps_tr")
                        nc.tensor.transpose(
                            pt2[:, :D],
                            av_sb[:, gq * P : (gq + 1) * P],
                            ident[:D, :D],
                        )
                        vo_sb = vo_pool.tile([P, D], f32, tag="vo_sb")
                        nc.vector.tensor_copy(vo_sb, pt2[:, :D])
                        qs = (g * GRP + gq) * P
                        nc.sync.dma_start(out=outv[h, qs : qs + P, :], in_=vo_sb)

    legalize_waits(nc)
    return nc


def kernel(Q, K, V, attn_mask):
    Q = np.ascontiguousarray(np.asarray(Q), dtype=np.float32)
    K = np.ascontiguousarray(np.asarray(K), dtype=np.float32)
    V = np.ascontiguousarray(np.asarray(V), dtype=np.float32)
    M = np.asarray(attn_mask).astype(np.uint8)

    qf = Q.reshape(B * H, S, D)
    kf = K.reshape(B * H, S, D)
    vf = V.reshape(B * H, S, D)
    mf = M.reshape(B * H, S, S)

    if "nc" not in _COMPILED:
        _COMPILED["nc"] = build_bass()
    nc = _COMPILED["nc"]

    in_maps = []
    for c in range(N_CORES):
        sl = slice(c * HEADS_PER_CORE, (c + 1) * HEADS_PER_CORE)
        in_maps.append(
            {
                "q": np.ascontiguousarray(qf[sl]),
                "k": np.ascontiguousarray(kf[sl]),
                "v": np.ascontiguousarray(vf[sl]),
                "m": np.ascontiguousarray(mf[sl]),
            }
        )

    res = run_bass_kernel_spmd(nc, in_maps, core_ids=list(range(N_CORES)))
    results = res.results

    attn_v = np.concatenate([results[c]["out_v"] for c in range(N_CORES)], axis=0)
    attn_p = np.concatenate([results[c]["out_p"] for c in range(N_CORES)], axis=0)
    attn_v = attn_v.reshape(B, H, S, D).astype(np.float32)
    attn_p = attn_p.reshape(B, H, S, S).astype(np.float32)
    return attn_v, attn_p


if __name__ == "__main__":
    rng = np.random.default_rng(0)
    Q = rng.standard_normal((B, H, S, D), dtype=np.float32)
    K = rng.standard_normal((B, H, S, D), dtype=np.float32)
    V = rng.standard_normal((B, H, S, D), dtype=np.float32)
    Mm = rng.integers(0, 2, size=(B, H, S, S)).astype(bool)
    av, ap = kernel(Q, K, V, Mm)
    print(av.shape, ap.shape, av.dtype, ap.dtype)
